# revision 31
# baseline (speedup 1.0000x reference)
"""Distributed exact kNN (EpisodicStore) on 8 Trainium2 NeuronCores.

Strategy:
  - Shard keys row-wise: 8 x 25000. Host pre-transposes each shard to
    [128(d), 25088] bf16 (zero-padded to 49*512 columns).
  - Device (per core): bf16 matmul scores = qT.T @ keysT into PSUM f32
    (512-col chunks), drain PSUM via mixed ACT-copy / DVE-fold into a
    bf16 pairwise-max fold tree 25088 -> 12544 -> 6272 -> 3136 -> 1568,
    add position jitter (iota * 1e-4, f32) to break bf16 ties, then DVE
    max / max_index -> per-core top-8 folded candidates per query.
  - Host: expand each folded candidate to its 16 source columns,
    exact-rescore all candidate rows in f64, global top-k, gather slots.

  The fold tree + exact host rescore makes the bf16 score precision
  irrelevant for correctness: the candidate set provably contains the
  true top-8 (top-8 of block-maxima covers all blocks containing top-8
  elements), and the final selection uses f64 scores.
"""

import numpy as np
import ml_dtypes
from contextlib import ExitStack

import concourse.bass as bass
import concourse.mybir as mybir
import concourse.tile as tile
from concourse.bass_utils import run_bass_kernel_spmd

# Problem shapes (hardcoded per contest contract)
B, N, D, S = 256, 200000, 128, 256
NCORES = 8
NLOC = N // NCORES            # 25000
NPAD = 25088                  # 49 * 512
L1W = NPAD // 2               # 12544
L4W = NPAD // 16              # 1568
JITTER = 1e-4

f32 = mybir.dt.float32
bf16 = mybir.dt.bfloat16
u16 = mybir.dt.uint16
i32 = mybir.dt.int32

_cache: dict = {}

# drain strategy pattern: nA full-copy groups out of 24 (rest are
# half-copy + psum-fold). Interleave A/B to keep both engines busy.
_NA = 10
_NO_TAIL = False          # experiment: skip fold2+/jitter/max/idx
_JITTER_ENGINE = "vector"  # "vector" | "gpsimd" (walrus rejects TensorScalarPtr on Pool)
_DEDUPE_LDW = True        # drop back-to-back identical weight loads
_STRICT = True            # raise if ISA 1-wait limit still violated


def _strategy(i):
    # spread nA "A" groups evenly among 24
    return "A" if (i * _NA) % 24 < _NA else "B"


def _build():
    nc = bass.Bass()
    qT = nc.declare_dram_parameter("qT", [D, B], bf16, isOutput=False)
    kT = nc.declare_dram_parameter("kT", [D, NPAD], bf16, isOutput=False)
    ovals = nc.declare_dram_parameter("ovals", [B, 8], f32, isOutput=True)
    oidx = nc.declare_dram_parameter("oidx", [B, 8], u16, isOutput=True)

    with tile.TileContext(nc) as tc, ExitStack() as ctx:
        # Separate PSUM pools so each tile has exactly ONE reader engine
        # (walrus allows only one sync-wait on a Matmult instruction).
        # 8 banks total: psa [128,1024]x2 (ACT-read) + psd [128,1024]x2
        # (DVE-read).
        psa = ctx.enter_context(tc.tile_pool(name="psa", bufs=2, space="PSUM"))
        psd = ctx.enter_context(tc.tile_pool(name="psd", bufs=2, space="PSUM"))
        l0pool = ctx.enter_context(tc.tile_pool(name="l0", bufs=6))
        hpool = ctx.enter_context(tc.tile_pool(name="h", bufs=6))
        ktpool = ctx.enter_context(tc.tile_pool(name="ktp", bufs=2))
        persist = ctx.enter_context(tc.tile_pool(name="persist", bufs=1))

        qt = persist.tile([D, B], bf16, name="qt", tag="qt")
        nc.sync.dma_start(qt[:], qT[:])

        l1 = persist.tile([128, 2, L1W], bf16, name="l1", tag="l1")
        l2 = persist.tile([128, 2, L1W // 2], bf16, name="l2", tag="l2")
        l3 = persist.tile([128, 2, L1W // 4], bf16, name="l3", tag="l3")
        l4 = persist.tile([128, 2, L4W], bf16, name="l4", tag="l4")
        l4j = persist.tile([128, 2, L4W], f32, name="l4j", tag="l4j")
        iota_i = persist.tile([128, L4W], i32, name="iota_i", tag="ioi")
        iota_f = persist.tile([128, L4W], f32, name="iota_f", tag="iof")
        vals = persist.tile([128, 2, 8], f32, name="vals", tag="vals")
        gp_scratch = persist.tile([128, 8], bf16, name="gp_scratch", tag="gpscr")
        idxs = persist.tile([128, 2, 8], u16, name="idxs", tag="idxs")

        # one-time iota -> f32 (scaled on the copy via tensor_scalar mult)
        nc.gpsimd.iota(iota_i[:], pattern=[[1, L4W]], base=0, channel_multiplier=0)
        nc.vector.tensor_scalar_mul(iota_f[:], iota_i[:], JITTER)
        # early DVE touch of iota_f: absorbs the Pool-sem wait so later DVE
        # jitter ops carry at most their (strippable) self wait
        dve_scratch = persist.tile([128, 8], f32, name="dve_scratch", tag="dvescr")
        nc.vector.tensor_copy(dve_scratch[:], iota_f[:, 0:8])

        # keysT strips, all resident in SBUF (49 KB/partition total).
        # qb is the OUTER loop so qb0's fold/max tail overlaps qb1's drains.
        strip_w = [2048, 2048, 4096, 4096, 4096, 4096, 4096, 512]
        strip_off = [0, 2048, 4096, 8192, 12288, 16384, 20480, 24576]
        strips = []
        for s, (soff, sw) in enumerate(zip(strip_off, strip_w)):
            kts = persist.tile([D, sw], bf16, tag=f"kt{s}", name=f"kt{s}")
            nc.sync.dma_start(kts[:], kT[:, soff:soff + sw])
            strips.append(kts)

        def drain_groups(qb, delay=0):
            """Emit matmul+drain per 2048 group; yields after each group.

            The DVE fold of group g is emitted `delay` groups later, so its
            ACT partner copy is already complete when the fold issues -- this
            removes the per-group ACT->DVE serialization chain.
            """
            qsl = qt[:, qb * 128:(qb + 1) * 128]
            gidx = 0
            consumers = []
            for s, (soff, sw) in enumerate(zip(strip_off, strip_w)):
                kts = strips[s]
                if qb == 0:
                    # PE-queue touch: absorbs the strip's DMA-sem wait so the
                    # first matmul carries only its psum-slot release wait
                    # (walrus MM struct allows a single sync-wait).
                    nc.tensor.ldweights(kts[:, 0:128])
                for gg in range(max(1, sw // 2048)):
                    gw = min(2048, sw)
                    base = soff + gg * 2048     # original column base
                    hw = gw // 2
                    l1sl = l1[:, qb, base // 2: base // 2 + hw]
                    koff = gg * 2048

                    def mm(dst, c0, ncols, kts=kts, koff=koff, qsl=qsl):
                        done = 0
                        while done < ncols:
                            w = min(512, ncols - done)
                            nc.tensor.matmul(
                                dst[:, done:done + w],
                                qsl,
                                kts[:, koff + c0 + done: koff + c0 + done + w],
                                start=True, stop=True,
                            )
                            done += w

                    if gw == 2048 and _strategy(gidx) == "A":
                        # full-copy drain: ACT copies both halves from two
                        # ACT-read psum tiles, DVE folds the SBUF copies (4x)
                        pa = psa.tile([128, 1024], f32, tag="psa", name="psa")
                        mm(pa, 0, hw)
                        l0a = l0pool.tile([128, 1024], bf16, tag="l0", name="l0a")
                        l0b = l0pool.tile([128, 1024], bf16, tag="l0", name="l0b")
                        nc.scalar.copy(l0a[:, :hw], pa[:, :hw])
                        pb = psa.tile([128, 1024], f32, tag="psa", name="psb")
                        mm(pb, hw, hw)
                        nc.scalar.copy(l0b[:, :hw], pb[:, :hw])
                        consumers.append(lambda l1sl=l1sl, l0a=l0a, l0b=l0b, hw=hw:
                                         nc.vector.tensor_max(l1sl, l0a[:, :hw], l0b[:, :hw]))
                    else:
                        # split drain: DVE folds the lo psum tile against
                        # ACT's copy of the hi tile
                        lo = psd.tile([128, hw], f32, tag="psd", name="psd")
                        mm(lo, 0, hw)
                        hi = psa.tile([128, hw], f32, tag="psa", name="psa")
                        mm(hi, hw, hw)
                        ht = hpool.tile([128, hw], bf16, tag="h", name="h")
                        nc.scalar.copy(ht[:, :hw], hi[:, :hw])
                        consumers.append(lambda l1sl=l1sl, lo=lo, ht=ht, hw=hw:
                                         nc.vector.tensor_max(l1sl, lo[:, :hw], ht[:, :hw]))
                    if gw == 2048:
                        gidx += 1
                    if len(consumers) > delay:
                        consumers.pop(0)()
                    yield
            while consumers:
                consumers.pop(0)()
                yield

        def tail_ops(qb, jitter_on_gp):
            """Closures for the fold/jitter/max tail, smallest-grain first
            pieces so they can interleave with the other qb's drains."""
            ops = []
            w2 = L1W // 2      # 6272
            w3 = w2 // 2       # 3136
            w4 = w3 // 2       # 1568
            for p in range(4):  # fold2 in 4 pieces of 1568
                a, b = p * (w2 // 4), (p + 1) * (w2 // 4)
                ops.append(lambda a=a, b=b: nc.vector.tensor_max(
                    l2[:, qb, a:b], l1[:, qb, a:b], l1[:, qb, w2 + a:w2 + b]))
            for p in range(2):  # fold3 in 2 pieces
                a, b = p * (w3 // 2), (p + 1) * (w3 // 2)
                ops.append(lambda a=a, b=b: nc.vector.tensor_max(
                    l3[:, qb, a:b], l2[:, qb, a:b], l2[:, qb, w3 + a:w3 + b]))
            ops.append(lambda: nc.vector.tensor_max(
                l4[:, qb, :], l3[:, qb, :w4], l3[:, qb, w4:]))
            eng = nc.gpsimd if jitter_on_gp else nc.vector
            if jitter_on_gp:
                # Pool-queue touch of l4: absorbs the DVE-sem wait so the
                # jitter op itself carries a single (Pool-self) wait.
                ops.append(lambda: nc.gpsimd.tensor_copy(
                    gp_scratch[:], l4[:, qb, 0:8]))
            ops.append(lambda: eng.scalar_tensor_tensor(
                l4j[:, qb, :], l4[:, qb, :], 1.0, iota_f[:],
                op0=mybir.AluOpType.mult, op1=mybir.AluOpType.add))
            ops.append(lambda: nc.vector.max(vals[:, qb, :], l4j[:, qb, :]))
            ops.append(lambda: nc.vector.max_index(
                idxs[:, qb, :], vals[:, qb, :], l4j[:, qb, :]))
            ops.append(lambda: nc.sync.dma_start(
                ovals[qb * 128:(qb + 1) * 128, :], vals[:, qb, :]))
            ops.append(lambda: nc.sync.dma_start(
                oidx[qb * 128:(qb + 1) * 128, :], idxs[:, qb, :]))
            return ops

        for _ in drain_groups(0):
            pass
        pending = [] if _NO_TAIL else tail_ops(0, _JITTER_ENGINE == "gpsimd")
        for i, _ in enumerate(drain_groups(1)):
            if i >= 1 and pending:
                pending.pop(0)()
        while pending:
            pending.pop(0)()
        if not _NO_TAIL:
            for op in tail_ops(1, False):
                op()

    if _DEDUPE_LDW:
        _dedupe_ldweights(nc)
    _strip_redundant_waits(nc)
    nc.finalize()
    return nc


def _dedupe_ldweights(nc):
    """Remove back-to-back InstLdweights with identical weight APs on the PE
    stream (the stationary tile is already loaded). Only drops instructions
    carrying no sync waits/updates."""
    f = nc.m.functions[0]
    for b in f.blocks:
        pe = [i for i in b.instructions if str(i.engine) == "EngineType.PE"]
        drop = []
        last_w = None
        for inst in pe:
            nm = type(inst).__name__
            if nm == "InstLdweights":
                key = str(inst.ins[0]) if inst.ins else None
                si = inst.sync_info
                clean = si is None or (not si.on_wait and not si.on_update)
                if key is not None and key == last_w and clean:
                    drop.append(inst)
                else:
                    last_w = key
            elif nm == "InstMatmult":
                pass  # keeps loaded weights
            else:
                last_w = None  # unknown PE op: be safe
        names = {i.name for i in drop}
        if not names:
            continue
        keep = [i for i in b.instructions if i.name not in names]
        # rebuild block instruction list in place
        while len(b.instructions):
            b.instructions.pop()
        for i in keep:
            b.instructions.append(i)
        for n in names:
            nc.inst_map.pop(n, None)


def _strip_redundant_waits(nc, iters=3, verbose=False):
    """Drop sync waits that are transitively guaranteed by another wait on
    the same instruction.

    Tile's sem assignment is per-proc minimal but not transitively minimal
    (documented limitation), and several walrus ISA structs (Matmult,
    TensorTensor, ...) only support a single sync-wait. Proof used: engine
    E executes its stream in order, and an instruction's updates fire after
    its waits are satisfied and all earlier E-instructions completed. So
    when sem S (single-updater engine E) reaches value v, every wait seen
    earlier in E's stream is satisfied and every earlier update by E has
    fired. guarantee[(S, v)] records those floors; a wait W2 on an
    instruction that also waits W1 can be dropped if
    guarantee[(W1.sem, W1.value)][W2.sem] >= W2.value.
    """
    f = nc.m.functions[0]
    sem_updaters: dict = {}
    streams: dict = {}
    for b in f.blocks:
        for inst in b.instructions:
            streams.setdefault(str(inst.engine), []).append(inst)
            si = inst.sync_info
            if si:
                for u in si.on_update:
                    if u.ant_name:
                        sem_updaters.setdefault(u.ant_name, set()).add(str(inst.engine))
    single = {s for s, e in sem_updaters.items() if len(e) == 1}

    guarantee: dict = {}
    for _ in range(iters):
        for eng, insts in streams.items():
            floors: dict = {}
            for inst in insts:
                si = inst.sync_info
                if not si:
                    continue
                for w in si.on_wait:
                    if w.ant_name in single and w.wait_value is not None and w.wait_reg is None:
                        if floors.get(w.ant_name, -1) < w.wait_value:
                            floors[w.ant_name] = w.wait_value
                        # merge what that sem value itself guarantees
                        g = guarantee.get((w.ant_name, w.wait_value))
                        if g:
                            for s2, v2 in g.items():
                                if floors.get(s2, -1) < v2:
                                    floors[s2] = v2
                for u in si.on_update:
                    if u.ant_name in single:
                        v = floors.get(u.ant_name, 0) + (u.update_value or 1)
                        # floors snapshot excludes this instruction's own
                        # updates (taken before applying them)
                        guarantee[(u.ant_name, v)] = dict(floors)
                # apply own updates to floors after snapshotting
                for u in si.on_update:
                    if u.ant_name in single:
                        floors[u.ant_name] = floors.get(u.ant_name, 0) + (u.update_value or 1)

    n_stripped = 0
    worst = {}
    for eng, insts in streams.items():
        for inst in insts:
            si = inst.sync_info
            if not si or len(si.on_wait) <= 1:
                continue
            waits = list(si.on_wait)
            keep: list = []
            for i, w in enumerate(waits):
                covered = False
                if w.ant_name in single and w.wait_reg is None:
                    # evidence: kept waits + not-yet-processed waits (sound
                    # even if those are later dropped, since dropping requires
                    # a kept wait that implies them)
                    for ow in keep + waits[i + 1:]:
                        if ow.ant_name not in single or ow.wait_reg is not None:
                            continue
                        g = guarantee.get((ow.ant_name, ow.wait_value))
                        if g and g.get(w.ant_name, -1) >= (w.wait_value or 0):
                            covered = True
                            break
                if covered:
                    n_stripped += 1
                else:
                    keep.append(w)
            if len(keep) < len(waits):
                inst.sync_info = type(si)(on_wait=keep, on_update=list(si.on_update))
            if len(keep) > 1:
                worst.setdefault(type(inst).__name__, []).append(
                    (inst.name, [(w.ant_name, w.wait_value) for w in keep]))
    if verbose:
        print(f"stripped {n_stripped} redundant waits; multi-wait left: "
              f"{ {k: len(v) for k, v in worst.items()} }")
    bad = [x for k, v in worst.items() for x in v
           if k not in ("InstDMACopy",)]
    if bad and _STRICT:
        raise RuntimeError(f"instructions still over the 1-wait ISA limit: {bad[:5]}")


def _get_nc():
    if "nc" not in _cache:
        _cache["nc"] = _build()
    return _cache["nc"]


def _expand_l4(pos):
    """pos: [...] int array of L4 positions -> [..., 16] original columns."""
    pos = np.asarray(pos, dtype=np.int64)
    p_l1 = pos[..., None] + L4W * np.arange(8, dtype=np.int64)  # [..., 8]
    big = p_l1 < 12288
    g = p_l1 // 1024
    i = p_l1 % 1024
    o1 = np.where(big, 2048 * g + i, 24576 + (p_l1 - 12288))
    off = np.where(big, 1024, 256)
    return np.concatenate([o1, o1 + off], axis=-1)  # [..., 16]


def kernel(query, keys, slots, k):
    query = np.asarray(query, dtype=np.float32)
    keys = np.asarray(keys, dtype=np.float32)
    slots = np.asarray(slots, dtype=np.float32)
    kk = int(k)
    assert 1 <= kk <= 8

    qT = np.ascontiguousarray(query.T).astype(ml_dtypes.bfloat16)

    in_maps = []
    for c in range(NCORES):
        shard = keys[c * NLOC:(c + 1) * NLOC]
        kT = np.zeros((D, NPAD), dtype=ml_dtypes.bfloat16)
        kT[:, :NLOC] = np.ascontiguousarray(shard.T).astype(ml_dtypes.bfloat16)
        in_maps.append({"qT": qT, "kT": kT})

    nc = _get_nc()
    import os
    trace = bool(int(os.environ.get("KNN_TRACE", "0")))
    res = run_bass_kernel_spmd(nc, in_maps, list(range(NCORES)), trace=trace)
    _cache["last_results"] = res

    # Gather candidates: per core, [B, 8] folded positions -> 16 origins each
    all_rows = []
    for c in range(NCORES):
        pos = res.results[c]["oidx"].astype(np.int64)          # [B, 8]
        origs = _expand_l4(pos).reshape(B, -1)                 # [B, 128]
        rows = np.where(origs < NLOC, origs + c * NLOC, -1)
        all_rows.append(rows)
    cand = np.concatenate(all_rows, axis=1)                    # [B, 1024]

    # Exact rescore in f64 (batched) then per-query top-k
    safe = np.maximum(cand, 0)
    Kg = keys[safe]                                            # [B, 1024, D] f32
    sc = np.matmul(Kg.astype(np.float64), query[:, :, None].astype(np.float64))[..., 0]
    sc = np.where(cand < 0, -np.inf, sc)                       # [B, 1024]

    top_idx = np.empty((B, kk), dtype=np.int64)
    top_scores = np.empty((B, kk), dtype=np.float32)
    for b in range(B):
        u, ui = np.unique(cand[b], return_index=True)
        if u[0] < 0:
            u, ui = u[1:], ui[1:]
        s_u = sc[b, ui]
        order = np.argsort(-s_u, kind="stable")[:kk]
        top_idx[b] = u[order]
        top_scores[b] = s_u[order].astype(np.float32)

    retrieved = slots[top_idx]                                 # [B, k, S]
    return retrieved, top_scores


# revision 36
# speedup vs baseline: 1.0123x; 1.0123x over previous
"""Distributed exact kNN (EpisodicStore) on 8 Trainium2 NeuronCores.

Strategy:
  - Shard keys row-wise: 8 x 25000. Host pre-transposes each shard to
    [128(d), 25088] bf16 (zero-padded to 49*512 columns).
  - Device (per core): bf16 matmul scores = qT.T @ keysT into PSUM f32
    (512-col chunks), drain PSUM via mixed ACT-copy / DVE-fold into a
    bf16 pairwise-max fold tree 25088 -> 12544 -> 6272 -> 3136 -> 1568,
    add position jitter (iota * 1e-4, f32) to break bf16 ties, then DVE
    max / max_index -> per-core top-8 folded candidates per query.
  - Host: expand each folded candidate to its 16 source columns,
    exact-rescore all candidate rows in f64, global top-k, gather slots.

  The fold tree + exact host rescore makes the bf16 score precision
  irrelevant for correctness: the candidate set provably contains the
  true top-8 (top-8 of block-maxima covers all blocks containing top-8
  elements), and the final selection uses f64 scores.
"""

import numpy as np
import ml_dtypes
from contextlib import ExitStack

import concourse.bass as bass
import concourse.mybir as mybir
import concourse.tile as tile
from concourse.bass_utils import run_bass_kernel_spmd

# Problem shapes (hardcoded per contest contract)
B, N, D, S = 256, 200000, 128, 256
NCORES = 8
NLOC = N // NCORES            # 25000
NPAD = 25088                  # 49 * 512
L1W = NPAD // 2               # 12544
L4W = NPAD // 16              # 1568
JITTER = 1e-4

f32 = mybir.dt.float32
bf16 = mybir.dt.bfloat16
u16 = mybir.dt.uint16
i32 = mybir.dt.int32

_cache: dict = {}

# drain strategy per 12-group qb pass: positions 0-1 path-A (primes the
# psa recycle chain for wait proofs), last _ND//2 path-D (DVE-only drain,
# lets ACT finish early), middle interleaves A/B to _NA total A-groups.
_NA = 10
_ND = 4
_NO_TAIL = False          # experiment: skip fold2+/jitter/max/idx
_JITTER_ENGINE = "vector"  # "vector" | "gpsimd" (walrus rejects TensorScalarPtr on Pool)
_DEDUPE_LDW = True        # drop back-to-back identical weight loads
_STRICT = True            # raise if ISA 1-wait limit still violated


def _strategy(i):
    j = i % 12
    nd2 = _ND // 2
    if j >= 12 - nd2:
        return "D"
    if j < 2:
        return "A"
    # middle 10-nd2 slots: need _NA//2 - 2 more A's, spread evenly
    need_a = _NA // 2 - 2
    nmid = 10 - nd2
    return "A" if ((j - 2) * need_a) % nmid < need_a else "B"


def _build():
    nc = bass.Bass()
    qT = nc.declare_dram_parameter("qT", [D, B], bf16, isOutput=False)
    kT = nc.declare_dram_parameter("kT", [D, NPAD], bf16, isOutput=False)
    ovals = nc.declare_dram_parameter("ovals", [B, 8], f32, isOutput=True)
    oidx = nc.declare_dram_parameter("oidx", [B, 8], u16, isOutput=True)

    with tile.TileContext(nc) as tc, ExitStack() as ctx:
        # Separate PSUM pools so each tile has exactly ONE reader engine
        # (walrus allows only one sync-wait on a Matmult instruction).
        # 8 banks total: psa [128,1024]x2 (ACT-read) + psd [128,1024]x2
        # (DVE-read).
        psa = ctx.enter_context(tc.tile_pool(name="psa", bufs=2, space="PSUM"))
        psd = ctx.enter_context(tc.tile_pool(name="psd", bufs=2, space="PSUM"))
        l0pool = ctx.enter_context(tc.tile_pool(name="l0", bufs=6))
        hpool = ctx.enter_context(tc.tile_pool(name="h", bufs=6))
        hdpool = ctx.enter_context(tc.tile_pool(name="hd", bufs=3))
        ktpool = ctx.enter_context(tc.tile_pool(name="ktp", bufs=2))
        persist = ctx.enter_context(tc.tile_pool(name="persist", bufs=1))

        qt = persist.tile([D, B], bf16, name="qt", tag="qt")
        nc.sync.dma_start(qt[:], qT[:])

        l1 = persist.tile([128, 2, L1W], bf16, name="l1", tag="l1")
        l2 = persist.tile([128, 2, L1W // 2], bf16, name="l2", tag="l2")
        l3 = persist.tile([128, 2, L1W // 4], bf16, name="l3", tag="l3")
        l4 = persist.tile([128, 2, L4W], bf16, name="l4", tag="l4")
        l4j = persist.tile([128, 2, L4W], f32, name="l4j", tag="l4j")
        iota_i = persist.tile([128, L4W], i32, name="iota_i", tag="ioi")
        iota_f = persist.tile([128, L4W], f32, name="iota_f", tag="iof")
        vals = persist.tile([128, 2, 8], f32, name="vals", tag="vals")
        gp_scratch = persist.tile([128, 8], bf16, name="gp_scratch", tag="gpscr")
        idxs = persist.tile([128, 2, 8], u16, name="idxs", tag="idxs")

        # one-time iota -> f32 (scaled on the copy via tensor_scalar mult)
        nc.gpsimd.iota(iota_i[:], pattern=[[1, L4W]], base=0, channel_multiplier=0)
        nc.vector.tensor_scalar_mul(iota_f[:], iota_i[:], JITTER)
        # early DVE touch of iota_f: absorbs the Pool-sem wait so later DVE
        # jitter ops carry at most their (strippable) self wait
        dve_scratch = persist.tile([128, 8], f32, name="dve_scratch", tag="dvescr")
        nc.vector.tensor_copy(dve_scratch[:], iota_f[:, 0:8])

        # keysT strips, all resident in SBUF (49 KB/partition total).
        # qb is the OUTER loop so qb0's fold/max tail overlaps qb1's drains.
        strip_w = [512, 2048, 2048, 4096, 4096, 4096, 4096, 4096]
        strip_off = [24576, 0, 2048, 4096, 8192, 12288, 16384, 20480]
        strips = []
        for s, (soff, sw) in enumerate(zip(strip_off, strip_w)):
            kts = persist.tile([D, sw], bf16, tag=f"kt{s}", name=f"kt{s}")
            nc.sync.dma_start(kts[:], kT[:, soff:soff + sw])
            strips.append(kts)

        def drain_groups(qb, delay=0):
            """Emit matmul+drain per 2048 group; yields after each group.

            The DVE fold of group g is emitted `delay` groups later, so its
            ACT partner copy is already complete when the fold issues -- this
            removes the per-group ACT->DVE serialization chain.
            """
            qsl = qt[:, qb * 128:(qb + 1) * 128]
            gidx = 0
            consumers = []
            for s, (soff, sw) in enumerate(zip(strip_off, strip_w)):
                kts = strips[s]
                if qb == 0:
                    # PE-queue touch: absorbs the strip's DMA-sem wait so the
                    # first matmul carries only its psum-slot release wait
                    # (walrus MM struct allows a single sync-wait).
                    nc.tensor.ldweights(kts[:, 0:128])
                for gg in range(max(1, sw // 2048)):
                    gw = min(2048, sw)
                    base = soff + gg * 2048     # original column base
                    hw = gw // 2
                    l1sl = l1[:, qb, base // 2: base // 2 + hw]
                    koff = gg * 2048

                    def mm(dst, c0, ncols, kts=kts, koff=koff, qsl=qsl):
                        done = 0
                        while done < ncols:
                            w = min(512, ncols - done)
                            nc.tensor.matmul(
                                dst[:, done:done + w],
                                qsl,
                                kts[:, koff + c0 + done: koff + c0 + done + w],
                                start=True, stop=True,
                            )
                            done += w

                    if gw == 2048 and _strategy(gidx) == "A":
                        # full-copy drain: ACT copies both halves from two
                        # ACT-read psum tiles, DVE folds the SBUF copies (4x)
                        pa = psa.tile([128, 1024], f32, tag="psa", name="psa")
                        mm(pa, 0, hw)
                        l0a = l0pool.tile([128, 1024], bf16, tag="l0", name="l0a")
                        l0b = l0pool.tile([128, 1024], bf16, tag="l0", name="l0b")
                        nc.scalar.copy(l0a[:, :hw], pa[:, :hw])
                        pb = psa.tile([128, 1024], f32, tag="psa", name="psb")
                        mm(pb, hw, hw)
                        nc.scalar.copy(l0b[:, :hw], pb[:, :hw])
                        consumers.append(lambda l1sl=l1sl, l0a=l0a, l0b=l0b, hw=hw:
                                         nc.vector.tensor_max(l1sl, l0a[:, :hw], l0b[:, :hw]))
                    elif gw == 2048 and _strategy(gidx) == "D":
                        # DVE-only drain: DVE copies hi psum to bf16 (own
                        # pool, so B-path ht slots keep single-writer ACT),
                        # then folds lo psum against it. No ACT involvement.
                        lo = psd.tile([128, hw], f32, tag="psd", name="psd")
                        mm(lo, 0, hw)
                        hi = psd.tile([128, hw], f32, tag="psd", name="psdh")
                        mm(hi, hw, hw)
                        hd = hdpool.tile([128, hw], bf16, tag="hd", name="hd")
                        nc.vector.tensor_copy(hd[:, :hw], hi[:, :hw])
                        consumers.append(lambda l1sl=l1sl, lo=lo, hd=hd, hw=hw:
                                         nc.vector.tensor_max(l1sl, lo[:, :hw], hd[:, :hw]))
                    else:
                        # split drain: DVE folds the lo psum tile against
                        # ACT's copy of the hi tile
                        lo = psd.tile([128, hw], f32, tag="psd", name="psd")
                        mm(lo, 0, hw)
                        hi = psa.tile([128, hw], f32, tag="psa", name="psa")
                        mm(hi, hw, hw)
                        ht = hpool.tile([128, hw], bf16, tag="h", name="h")
                        nc.scalar.copy(ht[:, :hw], hi[:, :hw])
                        consumers.append(lambda l1sl=l1sl, lo=lo, ht=ht, hw=hw:
                                         nc.vector.tensor_max(l1sl, lo[:, :hw], ht[:, :hw]))
                    if gw == 2048:
                        gidx += 1
                    if len(consumers) > delay:
                        consumers.pop(0)()
                    yield
            while consumers:
                consumers.pop(0)()
                yield

        def tail_ops(qb, jitter_on_gp):
            """Closures for the fold/jitter/max tail, smallest-grain first
            pieces so they can interleave with the other qb's drains."""
            ops = []
            w2 = L1W // 2      # 6272
            w3 = w2 // 2       # 3136
            w4 = w3 // 2       # 1568
            for p in range(4):  # fold2 in 4 pieces of 1568
                a, b = p * (w2 // 4), (p + 1) * (w2 // 4)
                ops.append(lambda a=a, b=b: nc.vector.tensor_max(
                    l2[:, qb, a:b], l1[:, qb, a:b], l1[:, qb, w2 + a:w2 + b]))
            for p in range(2):  # fold3 in 2 pieces
                a, b = p * (w3 // 2), (p + 1) * (w3 // 2)
                ops.append(lambda a=a, b=b: nc.vector.tensor_max(
                    l3[:, qb, a:b], l2[:, qb, a:b], l2[:, qb, w3 + a:w3 + b]))
            ops.append(lambda: nc.vector.tensor_max(
                l4[:, qb, :], l3[:, qb, :w4], l3[:, qb, w4:]))
            eng = nc.gpsimd if jitter_on_gp else nc.vector
            if jitter_on_gp:
                # Pool-queue touch of l4: absorbs the DVE-sem wait so the
                # jitter op itself carries a single (Pool-self) wait.
                ops.append(lambda: nc.gpsimd.tensor_copy(
                    gp_scratch[:], l4[:, qb, 0:8]))
            ops.append(lambda: eng.scalar_tensor_tensor(
                l4j[:, qb, :], l4[:, qb, :], 1.0, iota_f[:],
                op0=mybir.AluOpType.mult, op1=mybir.AluOpType.add))
            ops.append(lambda: nc.vector.max(vals[:, qb, :], l4j[:, qb, :]))
            ops.append(lambda: nc.vector.max_index(
                idxs[:, qb, :], vals[:, qb, :], l4j[:, qb, :]))
            ops.append(lambda: nc.sync.dma_start(
                ovals[qb * 128:(qb + 1) * 128, :], vals[:, qb, :]))
            ops.append(lambda: nc.sync.dma_start(
                oidx[qb * 128:(qb + 1) * 128, :], idxs[:, qb, :]))
            return ops

        for _ in drain_groups(0):
            pass
        pending = [] if _NO_TAIL else tail_ops(0, _JITTER_ENGINE == "gpsimd")
        for i, _ in enumerate(drain_groups(1)):
            if i >= 1 and pending:
                pending.pop(0)()
        while pending:
            pending.pop(0)()
        if not _NO_TAIL:
            for op in tail_ops(1, False):
                op()

    if _DEDUPE_LDW:
        _dedupe_ldweights(nc)
    _strip_redundant_waits(nc)
    nc.finalize()
    return nc


def _dedupe_ldweights(nc):
    """Remove back-to-back InstLdweights with identical weight APs on the PE
    stream (the stationary tile is already loaded). Only drops instructions
    carrying no sync waits/updates."""
    f = nc.m.functions[0]
    for b in f.blocks:
        pe = [i for i in b.instructions if str(i.engine) == "EngineType.PE"]
        drop = []
        last_w = None
        for inst in pe:
            nm = type(inst).__name__
            if nm == "InstLdweights":
                key = str(inst.ins[0]) if inst.ins else None
                si = inst.sync_info
                clean = si is None or (not si.on_wait and not si.on_update)
                if key is not None and key == last_w and clean:
                    drop.append(inst)
                else:
                    last_w = key
            elif nm == "InstMatmult":
                pass  # keeps loaded weights
            else:
                last_w = None  # unknown PE op: be safe
        names = {i.name for i in drop}
        if not names:
            continue
        keep = [i for i in b.instructions if i.name not in names]
        # rebuild block instruction list in place
        while len(b.instructions):
            b.instructions.pop()
        for i in keep:
            b.instructions.append(i)
        for n in names:
            nc.inst_map.pop(n, None)


def _strip_redundant_waits(nc, iters=3, verbose=False):
    """Drop sync waits that are transitively guaranteed by another wait on
    the same instruction.

    Tile's sem assignment is per-proc minimal but not transitively minimal
    (documented limitation), and several walrus ISA structs (Matmult,
    TensorTensor, ...) only support a single sync-wait. Proof used: engine
    E executes its stream in order, and an instruction's updates fire after
    its waits are satisfied and all earlier E-instructions completed. So
    when sem S (single-updater engine E) reaches value v, every wait seen
    earlier in E's stream is satisfied and every earlier update by E has
    fired. guarantee[(S, v)] records those floors; a wait W2 on an
    instruction that also waits W1 can be dropped if
    guarantee[(W1.sem, W1.value)][W2.sem] >= W2.value.
    """
    f = nc.m.functions[0]
    sem_updaters: dict = {}
    streams: dict = {}
    for b in f.blocks:
        for inst in b.instructions:
            streams.setdefault(str(inst.engine), []).append(inst)
            si = inst.sync_info
            if si:
                for u in si.on_update:
                    if u.ant_name:
                        sem_updaters.setdefault(u.ant_name, set()).add(str(inst.engine))
    single = {s for s, e in sem_updaters.items() if len(e) == 1}

    guarantee: dict = {}
    for _ in range(iters):
        for eng, insts in streams.items():
            floors: dict = {}
            for inst in insts:
                si = inst.sync_info
                if not si:
                    continue
                for w in si.on_wait:
                    if w.ant_name in single and w.wait_value is not None and w.wait_reg is None:
                        if floors.get(w.ant_name, -1) < w.wait_value:
                            floors[w.ant_name] = w.wait_value
                        # merge what that sem value itself guarantees
                        g = guarantee.get((w.ant_name, w.wait_value))
                        if g:
                            for s2, v2 in g.items():
                                if floors.get(s2, -1) < v2:
                                    floors[s2] = v2
                for u in si.on_update:
                    if u.ant_name in single:
                        v = floors.get(u.ant_name, 0) + (u.update_value or 1)
                        # floors snapshot excludes this instruction's own
                        # updates (taken before applying them)
                        guarantee[(u.ant_name, v)] = dict(floors)
                # apply own updates to floors after snapshotting
                for u in si.on_update:
                    if u.ant_name in single:
                        floors[u.ant_name] = floors.get(u.ant_name, 0) + (u.update_value or 1)

    n_stripped = 0
    worst = {}
    for eng, insts in streams.items():
        for inst in insts:
            si = inst.sync_info
            if not si or len(si.on_wait) <= 1:
                continue
            waits = list(si.on_wait)
            keep: list = []
            for i, w in enumerate(waits):
                covered = False
                if w.ant_name in single and w.wait_reg is None:
                    # evidence: kept waits + not-yet-processed waits (sound
                    # even if those are later dropped, since dropping requires
                    # a kept wait that implies them)
                    for ow in keep + waits[i + 1:]:
                        if ow.ant_name not in single or ow.wait_reg is not None:
                            continue
                        g = guarantee.get((ow.ant_name, ow.wait_value))
                        if g and g.get(w.ant_name, -1) >= (w.wait_value or 0):
                            covered = True
                            break
                if covered:
                    n_stripped += 1
                else:
                    keep.append(w)
            if len(keep) < len(waits):
                inst.sync_info = type(si)(on_wait=keep, on_update=list(si.on_update))
            if len(keep) > 1:
                worst.setdefault(type(inst).__name__, []).append(
                    (inst.name, [(w.ant_name, w.wait_value) for w in keep]))
    if verbose:
        print(f"stripped {n_stripped} redundant waits; multi-wait left: "
              f"{ {k: len(v) for k, v in worst.items()} }")
    bad = [x for k, v in worst.items() for x in v
           if k not in ("InstDMACopy",)]
    if bad and _STRICT:
        raise RuntimeError(f"instructions still over the 1-wait ISA limit: {bad[:5]}")


def _get_nc():
    if "nc" not in _cache:
        _cache["nc"] = _build()
    return _cache["nc"]


def _expand_l4(pos):
    """pos: [...] int array of L4 positions -> [..., 16] original columns."""
    pos = np.asarray(pos, dtype=np.int64)
    p_l1 = pos[..., None] + L4W * np.arange(8, dtype=np.int64)  # [..., 8]
    big = p_l1 < 12288
    g = p_l1 // 1024
    i = p_l1 % 1024
    o1 = np.where(big, 2048 * g + i, 24576 + (p_l1 - 12288))
    off = np.where(big, 1024, 256)
    return np.concatenate([o1, o1 + off], axis=-1)  # [..., 16]


def _io_spec(nc):
    import concourse.mybir as mybir
    in_names, out_specs = [], []
    pn = nc.partition_id_tensor.name if nc.partition_id_tensor else None
    for alloc in nc.m.functions[0].allocations:
        if not isinstance(alloc, mybir.MemoryLocationSet):
            continue
        name = alloc.memorylocations[0].name
        if alloc.kind == "ExternalInput":
            if name != pn:
                in_names.append(name)
        elif alloc.kind == "ExternalOutput":
            out_specs.append((name, tuple(alloc.tensor_shape),
                              mybir.dt.np(alloc.dtype)))
    return in_names, out_specs


def _run_device(nc, in_maps, trace):
    """Run the SPMD kernel; cache the jitted executable so repeat calls skip
    the per-call jax re-jit that run_bass_via_pjrt otherwise performs."""
    import jax

    class _R:  # minimal stand-in for BassKernelResults on the cached path
        def __init__(self, results):
            self.results = results
            self.exec_time_ns = None
            self.mean_exec_time_ns = None
            self.max_exec_time_core_id = None

    if not trace and "fn" in _cache:
        fn = _cache["fn"]
        in_names, out_specs = _cache["iospec"]
        concat_in = [np.concatenate([m[n] for m in in_maps], axis=0)
                     for n in in_names]
        zeros = [np.zeros((NCORES * s[0], *s[1:]), d) for _, s, d in out_specs]
        out = fn(*concat_in, *zeros)
        results = [
            {name: np.asarray(out[i]).reshape(NCORES, *shape)[c]
             for i, (name, shape, _) in enumerate(out_specs)}
            for c in range(NCORES)
        ]
        return _R(results)

    captured = {}
    orig_jit = jax.jit

    def spy(f, **kw):
        g = orig_jit(f, **kw)
        if "donate_argnums" in kw:
            captured["fn"] = g
        return g

    jax.jit = spy
    try:
        res = run_bass_kernel_spmd(nc, in_maps, list(range(NCORES)), trace=trace)
    finally:
        jax.jit = orig_jit
    if not trace and "fn" in captured:
        _cache["fn"] = captured["fn"]
        _cache["iospec"] = _io_spec(nc)
    return res


def kernel(query, keys, slots, k):
    query = np.asarray(query, dtype=np.float32)
    keys = np.asarray(keys, dtype=np.float32)
    slots = np.asarray(slots, dtype=np.float32)
    kk = int(k)
    assert 1 <= kk <= 8

    qT = np.ascontiguousarray(query.T).astype(ml_dtypes.bfloat16)

    kbf = keys.astype(ml_dtypes.bfloat16)
    in_maps = []
    for c in range(NCORES):
        kT = np.zeros((D, NPAD), dtype=ml_dtypes.bfloat16)
        kT[:, :NLOC] = np.ascontiguousarray(kbf[c * NLOC:(c + 1) * NLOC].T)
        in_maps.append({"qT": qT, "kT": kT})

    nc = _get_nc()
    import os
    trace = bool(int(os.environ.get("KNN_TRACE", "0")))
    res = _run_device(nc, in_maps, trace)
    _cache["last_results"] = res

    # Gather candidates: per core, [B, 8] folded positions -> 16 origins each
    all_rows = []
    for c in range(NCORES):
        pos = res.results[c]["oidx"].astype(np.int64)          # [B, 8]
        origs = _expand_l4(pos).reshape(B, -1)                 # [B, 128]
        rows = np.where(origs < NLOC, origs + c * NLOC, -1)
        all_rows.append(rows)
    cand = np.concatenate(all_rows, axis=1)                    # [B, 1024]

    # Exact rescore in f64 (batched) then per-query top-k
    safe = np.maximum(cand, 0)
    Kg = keys[safe]                                            # [B, 1024, D] f32
    sc = np.matmul(Kg.astype(np.float64), query[:, :, None].astype(np.float64))[..., 0]
    sc = np.where(cand < 0, -np.inf, sc)                       # [B, 1024]

    top_idx = np.empty((B, kk), dtype=np.int64)
    top_scores = np.empty((B, kk), dtype=np.float32)
    for b in range(B):
        u, ui = np.unique(cand[b], return_index=True)
        if u[0] < 0:
            u, ui = u[1:], ui[1:]
        s_u = sc[b, ui]
        order = np.argsort(-s_u, kind="stable")[:kk]
        top_idx[b] = u[order]
        top_scores[b] = s_u[order].astype(np.float32)

    retrieved = slots[top_idx]                                 # [B, k, S]
    return retrieved, top_scores


# revision 37
# speedup vs baseline: 1.0787x; 1.0656x over previous
"""Distributed exact kNN (EpisodicStore) on 8 Trainium2 NeuronCores.

Strategy:
  - Shard keys row-wise: 8 x 25000. Host pre-transposes each shard to
    [128(d), 25088] bf16 (zero-padded to 49*512 columns).
  - Device (per core): bf16 matmul scores = qT.T @ keysT into PSUM f32
    (512-col chunks), drain PSUM via mixed ACT-copy / DVE-fold into a
    bf16 pairwise-max fold tree 25088 -> 12544 -> 6272 -> 3136 -> 1568,
    add position jitter (iota * 1e-4, f32) to break bf16 ties, then DVE
    max / max_index -> per-core top-8 folded candidates per query.
  - Host: expand each folded candidate to its 16 source columns,
    exact-rescore all candidate rows in f64, global top-k, gather slots.

  The fold tree + exact host rescore makes the bf16 score precision
  irrelevant for correctness: the candidate set provably contains the
  true top-8 (top-8 of block-maxima covers all blocks containing top-8
  elements), and the final selection uses f64 scores.
"""

import numpy as np
import ml_dtypes
from contextlib import ExitStack

import concourse.bass as bass
import concourse.mybir as mybir
import concourse.tile as tile
from concourse.bass_utils import run_bass_kernel_spmd

# Problem shapes (hardcoded per contest contract)
B, N, D, S = 256, 200000, 128, 256
NCORES = 8
NLOC = N // NCORES            # 25000
NPAD = 25088                  # 49 * 512
L1W = NPAD // 2               # 12544
L4W = NPAD // 16              # 1568
L5W = NPAD // 32              # 784
JITTER = 1e-4

f32 = mybir.dt.float32
bf16 = mybir.dt.bfloat16
u16 = mybir.dt.uint16
i32 = mybir.dt.int32

_cache: dict = {}

# drain strategy per 12-group qb pass: positions 0-1 path-A (primes the
# psa recycle chain for wait proofs), last _ND//2 path-D (DVE-only drain,
# lets ACT finish early), middle interleaves A/B to _NA total A-groups.
_NA = 10
_ND = 4
_NO_TAIL = False          # experiment: skip fold2+/jitter/max/idx
_JITTER_ENGINE = "vector"  # "vector" | "gpsimd" (walrus rejects TensorScalarPtr on Pool)
_DEDUPE_LDW = True        # drop back-to-back identical weight loads
_STRICT = True            # raise if ISA 1-wait limit still violated


def _strategy(i):
    j = i % 12
    nd2 = _ND // 2
    if j >= 12 - nd2:
        return "D"
    if j < 2:
        return "A"
    # middle 10-nd2 slots: need _NA//2 - 2 more A's, spread evenly
    need_a = _NA // 2 - 2
    nmid = 10 - nd2
    return "A" if ((j - 2) * need_a) % nmid < need_a else "B"


def _build():
    nc = bass.Bass()
    qT = nc.declare_dram_parameter("qT", [D, B], bf16, isOutput=False)
    kT = nc.declare_dram_parameter("kT", [D, NPAD], bf16, isOutput=False)
    ovals = nc.declare_dram_parameter("ovals", [B, 8], f32, isOutput=True)
    oidx = nc.declare_dram_parameter("oidx", [B, 8], u16, isOutput=True)

    with tile.TileContext(nc) as tc, ExitStack() as ctx:
        # Separate PSUM pools so each tile has exactly ONE reader engine
        # (walrus allows only one sync-wait on a Matmult instruction).
        # 8 banks total: psa [128,1024]x2 (ACT-read) + psd [128,1024]x2
        # (DVE-read).
        psa = ctx.enter_context(tc.tile_pool(name="psa", bufs=2, space="PSUM"))
        psd = ctx.enter_context(tc.tile_pool(name="psd", bufs=2, space="PSUM"))
        l0pool = ctx.enter_context(tc.tile_pool(name="l0", bufs=6))
        hpool = ctx.enter_context(tc.tile_pool(name="h", bufs=6))
        hdpool = ctx.enter_context(tc.tile_pool(name="hd", bufs=3))
        ktpool = ctx.enter_context(tc.tile_pool(name="ktp", bufs=2))
        persist = ctx.enter_context(tc.tile_pool(name="persist", bufs=1))

        qt = persist.tile([D, B], bf16, name="qt", tag="qt")
        nc.sync.dma_start(qt[:], qT[:])

        l1 = persist.tile([128, 2, L1W], bf16, name="l1", tag="l1")
        l2 = persist.tile([128, 2, L1W // 2], bf16, name="l2", tag="l2")
        l3 = persist.tile([128, 2, L1W // 4], bf16, name="l3", tag="l3")
        l4 = persist.tile([128, 2, L4W], bf16, name="l4", tag="l4")
        l5 = persist.tile([128, 2, L5W], bf16, name="l5", tag="l5")
        l5j = persist.tile([128, 2, L5W], f32, name="l5j", tag="l5j")
        iota_i = persist.tile([128, L5W], i32, name="iota_i", tag="ioi")
        iota_f = persist.tile([128, L5W], f32, name="iota_f", tag="iof")
        vals = persist.tile([128, 2, 8], f32, name="vals", tag="vals")
        gp_scratch = persist.tile([128, 8], bf16, name="gp_scratch", tag="gpscr")
        idxs = persist.tile([128, 2, 8], u16, name="idxs", tag="idxs")

        # one-time iota -> f32 (scaled on the copy via tensor_scalar mult)
        nc.gpsimd.iota(iota_i[:], pattern=[[1, L5W]], base=0, channel_multiplier=0)
        nc.vector.tensor_scalar_mul(iota_f[:], iota_i[:], JITTER)
        # early DVE touch of iota_f: absorbs the Pool-sem wait so later DVE
        # jitter ops carry at most their (strippable) self wait
        dve_scratch = persist.tile([128, 8], f32, name="dve_scratch", tag="dvescr")
        nc.vector.tensor_copy(dve_scratch[:], iota_f[:, 0:8])

        # keysT strips, all resident in SBUF (49 KB/partition total).
        # qb is the OUTER loop so qb0's fold/max tail overlaps qb1's drains.
        strip_w = [512, 2048, 2048, 4096, 4096, 4096, 4096, 4096]
        strip_off = [24576, 0, 2048, 4096, 8192, 12288, 16384, 20480]
        strips = []
        for s, (soff, sw) in enumerate(zip(strip_off, strip_w)):
            kts = persist.tile([D, sw], bf16, tag=f"kt{s}", name=f"kt{s}")
            nc.sync.dma_start(kts[:], kT[:, soff:soff + sw])
            strips.append(kts)

        def drain_groups(qb, delay=0):
            """Emit matmul+drain per 2048 group; yields after each group.

            The DVE fold of group g is emitted `delay` groups later, so its
            ACT partner copy is already complete when the fold issues -- this
            removes the per-group ACT->DVE serialization chain.
            """
            qsl = qt[:, qb * 128:(qb + 1) * 128]
            gidx = 0
            consumers = []
            for s, (soff, sw) in enumerate(zip(strip_off, strip_w)):
                kts = strips[s]
                if qb == 0:
                    # PE-queue touch: absorbs the strip's DMA-sem wait so the
                    # first matmul carries only its psum-slot release wait
                    # (walrus MM struct allows a single sync-wait).
                    nc.tensor.ldweights(kts[:, 0:128])
                for gg in range(max(1, sw // 2048)):
                    gw = min(2048, sw)
                    base = soff + gg * 2048     # original column base
                    hw = gw // 2
                    l1sl = l1[:, qb, base // 2: base // 2 + hw]
                    koff = gg * 2048

                    def mm(dst, c0, ncols, kts=kts, koff=koff, qsl=qsl):
                        done = 0
                        while done < ncols:
                            w = min(512, ncols - done)
                            nc.tensor.matmul(
                                dst[:, done:done + w],
                                qsl,
                                kts[:, koff + c0 + done: koff + c0 + done + w],
                                start=True, stop=True,
                            )
                            done += w

                    if gw == 2048 and _strategy(gidx) == "A":
                        # full-copy drain: ACT copies both halves from two
                        # ACT-read psum tiles, DVE folds the SBUF copies (4x)
                        pa = psa.tile([128, 1024], f32, tag="psa", name="psa")
                        mm(pa, 0, hw)
                        l0a = l0pool.tile([128, 1024], bf16, tag="l0", name="l0a")
                        l0b = l0pool.tile([128, 1024], bf16, tag="l0", name="l0b")
                        nc.scalar.copy(l0a[:, :hw], pa[:, :hw])
                        pb = psa.tile([128, 1024], f32, tag="psa", name="psb")
                        mm(pb, hw, hw)
                        nc.scalar.copy(l0b[:, :hw], pb[:, :hw])
                        consumers.append(lambda l1sl=l1sl, l0a=l0a, l0b=l0b, hw=hw:
                                         nc.vector.tensor_max(l1sl, l0a[:, :hw], l0b[:, :hw]))
                    elif gw == 2048 and _strategy(gidx) == "D":
                        # DVE-only drain: DVE copies hi psum to bf16 (own
                        # pool, so B-path ht slots keep single-writer ACT),
                        # then folds lo psum against it. No ACT involvement.
                        lo = psd.tile([128, hw], f32, tag="psd", name="psd")
                        mm(lo, 0, hw)
                        hi = psd.tile([128, hw], f32, tag="psd", name="psdh")
                        mm(hi, hw, hw)
                        hd = hdpool.tile([128, hw], bf16, tag="hd", name="hd")
                        nc.vector.tensor_copy(hd[:, :hw], hi[:, :hw])
                        consumers.append(lambda l1sl=l1sl, lo=lo, hd=hd, hw=hw:
                                         nc.vector.tensor_max(l1sl, lo[:, :hw], hd[:, :hw]))
                    else:
                        # split drain: DVE folds the lo psum tile against
                        # ACT's copy of the hi tile
                        lo = psd.tile([128, hw], f32, tag="psd", name="psd")
                        mm(lo, 0, hw)
                        hi = psa.tile([128, hw], f32, tag="psa", name="psa")
                        mm(hi, hw, hw)
                        ht = hpool.tile([128, hw], bf16, tag="h", name="h")
                        nc.scalar.copy(ht[:, :hw], hi[:, :hw])
                        consumers.append(lambda l1sl=l1sl, lo=lo, ht=ht, hw=hw:
                                         nc.vector.tensor_max(l1sl, lo[:, :hw], ht[:, :hw]))
                    if gw == 2048:
                        gidx += 1
                    if len(consumers) > delay:
                        consumers.pop(0)()
                    yield
            while consumers:
                consumers.pop(0)()
                yield

        def tail_ops(qb, jitter_on_gp):
            """Closures for the fold/jitter/max tail, smallest-grain first
            pieces so they can interleave with the other qb's drains."""
            ops = []
            w2 = L1W // 2      # 6272
            w3 = w2 // 2       # 3136
            w4 = w3 // 2       # 1568
            for p in range(4):  # fold2 in 4 pieces of 1568
                a, b = p * (w2 // 4), (p + 1) * (w2 // 4)
                ops.append(lambda a=a, b=b: nc.vector.tensor_max(
                    l2[:, qb, a:b], l1[:, qb, a:b], l1[:, qb, w2 + a:w2 + b]))
            for p in range(2):  # fold3 in 2 pieces
                a, b = p * (w3 // 2), (p + 1) * (w3 // 2)
                ops.append(lambda a=a, b=b: nc.vector.tensor_max(
                    l3[:, qb, a:b], l2[:, qb, a:b], l2[:, qb, w3 + a:w3 + b]))
            ops.append(lambda: nc.vector.tensor_max(
                l4[:, qb, :], l3[:, qb, :w4], l3[:, qb, w4:]))
            w5 = w4 // 2
            ops.append(lambda: nc.vector.tensor_max(
                l5[:, qb, :], l4[:, qb, :w5], l4[:, qb, w5:]))
            ops.append(lambda: nc.vector.scalar_tensor_tensor(
                l5j[:, qb, :], l5[:, qb, :], 1.0, iota_f[:],
                op0=mybir.AluOpType.mult, op1=mybir.AluOpType.add))
            ops.append(lambda: nc.vector.max(vals[:, qb, :], l5j[:, qb, :]))
            ops.append(lambda: nc.vector.max_index(
                idxs[:, qb, :], vals[:, qb, :], l5j[:, qb, :]))
            ops.append(lambda: nc.sync.dma_start(
                ovals[qb * 128:(qb + 1) * 128, :], vals[:, qb, :]))
            ops.append(lambda: nc.sync.dma_start(
                oidx[qb * 128:(qb + 1) * 128, :], idxs[:, qb, :]))
            return ops

        for _ in drain_groups(0):
            pass
        pending = [] if _NO_TAIL else tail_ops(0, _JITTER_ENGINE == "gpsimd")
        for i, _ in enumerate(drain_groups(1)):
            if i >= 1 and pending:
                pending.pop(0)()
        while pending:
            pending.pop(0)()
        if not _NO_TAIL:
            for op in tail_ops(1, False):
                op()

    if _DEDUPE_LDW:
        _dedupe_ldweights(nc)
    _strip_redundant_waits(nc)
    nc.finalize()
    return nc


def _dedupe_ldweights(nc):
    """Remove back-to-back InstLdweights with identical weight APs on the PE
    stream (the stationary tile is already loaded). Only drops instructions
    carrying no sync waits/updates."""
    f = nc.m.functions[0]
    for b in f.blocks:
        pe = [i for i in b.instructions if str(i.engine) == "EngineType.PE"]
        drop = []
        last_w = None
        for inst in pe:
            nm = type(inst).__name__
            if nm == "InstLdweights":
                key = str(inst.ins[0]) if inst.ins else None
                si = inst.sync_info
                clean = si is None or (not si.on_wait and not si.on_update)
                if key is not None and key == last_w and clean:
                    drop.append(inst)
                else:
                    last_w = key
            elif nm == "InstMatmult":
                pass  # keeps loaded weights
            else:
                last_w = None  # unknown PE op: be safe
        names = {i.name for i in drop}
        if not names:
            continue
        keep = [i for i in b.instructions if i.name not in names]
        # rebuild block instruction list in place
        while len(b.instructions):
            b.instructions.pop()
        for i in keep:
            b.instructions.append(i)
        for n in names:
            nc.inst_map.pop(n, None)


def _strip_redundant_waits(nc, iters=3, verbose=False):
    """Drop sync waits that are transitively guaranteed by another wait on
    the same instruction.

    Tile's sem assignment is per-proc minimal but not transitively minimal
    (documented limitation), and several walrus ISA structs (Matmult,
    TensorTensor, ...) only support a single sync-wait. Proof used: engine
    E executes its stream in order, and an instruction's updates fire after
    its waits are satisfied and all earlier E-instructions completed. So
    when sem S (single-updater engine E) reaches value v, every wait seen
    earlier in E's stream is satisfied and every earlier update by E has
    fired. guarantee[(S, v)] records those floors; a wait W2 on an
    instruction that also waits W1 can be dropped if
    guarantee[(W1.sem, W1.value)][W2.sem] >= W2.value.
    """
    f = nc.m.functions[0]
    sem_updaters: dict = {}
    streams: dict = {}
    for b in f.blocks:
        for inst in b.instructions:
            streams.setdefault(str(inst.engine), []).append(inst)
            si = inst.sync_info
            if si:
                for u in si.on_update:
                    if u.ant_name:
                        sem_updaters.setdefault(u.ant_name, set()).add(str(inst.engine))
    single = {s for s, e in sem_updaters.items() if len(e) == 1}

    guarantee: dict = {}
    for _ in range(iters):
        for eng, insts in streams.items():
            floors: dict = {}
            for inst in insts:
                si = inst.sync_info
                if not si:
                    continue
                for w in si.on_wait:
                    if w.ant_name in single and w.wait_value is not None and w.wait_reg is None:
                        if floors.get(w.ant_name, -1) < w.wait_value:
                            floors[w.ant_name] = w.wait_value
                        # merge what that sem value itself guarantees
                        g = guarantee.get((w.ant_name, w.wait_value))
                        if g:
                            for s2, v2 in g.items():
                                if floors.get(s2, -1) < v2:
                                    floors[s2] = v2
                for u in si.on_update:
                    if u.ant_name in single:
                        v = floors.get(u.ant_name, 0) + (u.update_value or 1)
                        # floors snapshot excludes this instruction's own
                        # updates (taken before applying them)
                        guarantee[(u.ant_name, v)] = dict(floors)
                # apply own updates to floors after snapshotting
                for u in si.on_update:
                    if u.ant_name in single:
                        floors[u.ant_name] = floors.get(u.ant_name, 0) + (u.update_value or 1)

    n_stripped = 0
    worst = {}
    for eng, insts in streams.items():
        for inst in insts:
            si = inst.sync_info
            if not si or len(si.on_wait) <= 1:
                continue
            waits = list(si.on_wait)
            keep: list = []
            for i, w in enumerate(waits):
                covered = False
                if w.ant_name in single and w.wait_reg is None:
                    # evidence: kept waits + not-yet-processed waits (sound
                    # even if those are later dropped, since dropping requires
                    # a kept wait that implies them)
                    for ow in keep + waits[i + 1:]:
                        if ow.ant_name not in single or ow.wait_reg is not None:
                            continue
                        g = guarantee.get((ow.ant_name, ow.wait_value))
                        if g and g.get(w.ant_name, -1) >= (w.wait_value or 0):
                            covered = True
                            break
                if covered:
                    n_stripped += 1
                else:
                    keep.append(w)
            if len(keep) < len(waits):
                inst.sync_info = type(si)(on_wait=keep, on_update=list(si.on_update))
            if len(keep) > 1:
                worst.setdefault(type(inst).__name__, []).append(
                    (inst.name, [(w.ant_name, w.wait_value) for w in keep]))
    if verbose:
        print(f"stripped {n_stripped} redundant waits; multi-wait left: "
              f"{ {k: len(v) for k, v in worst.items()} }")
    bad = [x for k, v in worst.items() for x in v
           if k not in ("InstDMACopy",)]
    if bad and _STRICT:
        raise RuntimeError(f"instructions still over the 1-wait ISA limit: {bad[:5]}")


def _get_nc():
    if "nc" not in _cache:
        _cache["nc"] = _build()
    return _cache["nc"]


def _expand_l4(pos):
    """pos: [...] int array of L5 positions -> [..., 32] original columns."""
    pos = np.asarray(pos, dtype=np.int64)
    p_l1 = pos[..., None] + L5W * np.arange(16, dtype=np.int64)  # [..., 16]
    big = p_l1 < 12288
    g = p_l1 // 1024
    i = p_l1 % 1024
    o1 = np.where(big, 2048 * g + i, 24576 + (p_l1 - 12288))
    off = np.where(big, 1024, 256)
    return np.concatenate([o1, o1 + off], axis=-1)  # [..., 16]


def _io_spec(nc):
    import concourse.mybir as mybir
    in_names, out_specs = [], []
    pn = nc.partition_id_tensor.name if nc.partition_id_tensor else None
    for alloc in nc.m.functions[0].allocations:
        if not isinstance(alloc, mybir.MemoryLocationSet):
            continue
        name = alloc.memorylocations[0].name
        if alloc.kind == "ExternalInput":
            if name != pn:
                in_names.append(name)
        elif alloc.kind == "ExternalOutput":
            out_specs.append((name, tuple(alloc.tensor_shape),
                              mybir.dt.np(alloc.dtype)))
    return in_names, out_specs


def _run_device(nc, in_maps, trace):
    """Run the SPMD kernel; cache the jitted executable so repeat calls skip
    the per-call jax re-jit that run_bass_via_pjrt otherwise performs."""
    import jax

    class _R:  # minimal stand-in for BassKernelResults on the cached path
        def __init__(self, results):
            self.results = results
            self.exec_time_ns = None
            self.mean_exec_time_ns = None
            self.max_exec_time_core_id = None

    if not trace and "fn" in _cache:
        fn = _cache["fn"]
        in_names, out_specs = _cache["iospec"]
        concat_in = [np.concatenate([m[n] for m in in_maps], axis=0)
                     for n in in_names]
        zeros = [np.zeros((NCORES * s[0], *s[1:]), d) for _, s, d in out_specs]
        out = fn(*concat_in, *zeros)
        results = [
            {name: np.asarray(out[i]).reshape(NCORES, *shape)[c]
             for i, (name, shape, _) in enumerate(out_specs)}
            for c in range(NCORES)
        ]
        return _R(results)

    captured = {}
    orig_jit = jax.jit

    def spy(f, **kw):
        g = orig_jit(f, **kw)
        if "donate_argnums" in kw:
            captured["fn"] = g
        return g

    jax.jit = spy
    try:
        res = run_bass_kernel_spmd(nc, in_maps, list(range(NCORES)), trace=trace)
    finally:
        jax.jit = orig_jit
    if not trace and "fn" in captured:
        _cache["fn"] = captured["fn"]
        _cache["iospec"] = _io_spec(nc)
    return res


def kernel(query, keys, slots, k):
    query = np.asarray(query, dtype=np.float32)
    keys = np.asarray(keys, dtype=np.float32)
    slots = np.asarray(slots, dtype=np.float32)
    kk = int(k)
    assert 1 <= kk <= 8

    qT = np.ascontiguousarray(query.T).astype(ml_dtypes.bfloat16)

    kbf = keys.astype(ml_dtypes.bfloat16)
    in_maps = []
    for c in range(NCORES):
        kT = np.zeros((D, NPAD), dtype=ml_dtypes.bfloat16)
        kT[:, :NLOC] = np.ascontiguousarray(kbf[c * NLOC:(c + 1) * NLOC].T)
        in_maps.append({"qT": qT, "kT": kT})

    nc = _get_nc()
    import os
    trace = bool(int(os.environ.get("KNN_TRACE", "0")))
    res = _run_device(nc, in_maps, trace)
    _cache["last_results"] = res

    # Gather candidates: per core, [B, 8] folded positions -> 16 origins each
    all_rows = []
    for c in range(NCORES):
        pos = res.results[c]["oidx"].astype(np.int64)          # [B, 8]
        origs = _expand_l4(pos).reshape(B, -1)                 # [B, 128]
        rows = np.where(origs < NLOC, origs + c * NLOC, -1)
        all_rows.append(rows)
    cand = np.concatenate(all_rows, axis=1)                    # [B, 1024]

    # Exact rescore in f64 (batched) then per-query top-k
    safe = np.maximum(cand, 0)
    Kg = keys[safe]                                            # [B, 1024, D] f32
    sc = np.matmul(Kg.astype(np.float64), query[:, :, None].astype(np.float64))[..., 0]
    sc = np.where(cand < 0, -np.inf, sc)                       # [B, 1024]

    top_idx = np.empty((B, kk), dtype=np.int64)
    top_scores = np.empty((B, kk), dtype=np.float32)
    for b in range(B):
        u, ui = np.unique(cand[b], return_index=True)
        if u[0] < 0:
            u, ui = u[1:], ui[1:]
        s_u = sc[b, ui]
        order = np.argsort(-s_u, kind="stable")[:kk]
        top_idx[b] = u[order]
        top_scores[b] = s_u[order].astype(np.float32)

    retrieved = slots[top_idx]                                 # [B, k, S]
    return retrieved, top_scores
